# revision 9
# baseline (speedup 1.0000x reference)
"""Trainium2 Bass kernel for nn_ClsCrossAttention (single-query CLS attention pooling).

Reference computation (per batch b, head h):
    tokens = features[b].reshape(C, H*W).T                  # [N=1024, C=768]
    K      = tokens @ W_k[h] + pos_embed                    # [N, 64]
    logits = K @ cls[h] / 8
    attn   = softmax(logits)
    out    = attn @ tokens                                  # [C]

Restructure (K is never materialized):
    logits[n, h] = tokens[n] . v_h + pos_bias[n, h]
        v_h      = W_k[h] @ cls[h] / 8          (host precomputed, [12, 768])
        pos_bias = pos_embed @ (cls/8)^T        (host precomputed, [1024, 12])
    Logits are ~+-0.02 so softmax needs no max subtraction. With d = exp(l)-1:
        out[h] = (colsum + d_h @ tokens) / (N + sum(d_h))
    colsum = sum_n tokens[n] computed exactly on the host in fp32.

Key PE trick: pooling needs tokens in n-major layout (PE transposes of the
c-major x tiles), and each transpose matmul already loads the x tile
[128c, 128n] as the stationary operand.  Widening the moving operand from
I(128) to [I | vT_k] (140 cols) makes the same matmuls also emit the
chunk-k logit partials in n-major — the separate logits/pos/dT matmuls all
disappear and the fused matmuls stream at ~61ns with LDWEIGHTS fully
hidden.  Per batch the PE runs just 48 fused matmuls + 16 pooling matmuls
(2 column-strip groups for array-level concurrency).

Drains: ACT/DVE alternate copying the transpose columns (PSUM->tok bf16)
and the 12 logit columns (PSUM->bf16 staging); the Pool engine (gpsimd,
which cannot read PSUM but can do SBUF tensor ops) then sums the 6 chunk
partials + pos bias with a small add tree, so no engine carries a serial
accumulation chain.  exp runs on the n-major [128, 96] logits so
d = exp(l)-1 is directly the pooling stationary operand.

DMA: features fp32 -> bf16 cast during DMA (SWDGE), ~3.1 MB fp32 read per
batch; 8 batches per core saturate the ~360 GB/s per-core HBM read budget,
which is the roofline for this kernel.
"""

import sys

sys.path.insert(0, "/opt/trn_rl_repo")

import numpy as np
import ml_dtypes

import concourse.bass as bass
import concourse.mybir as mybir
from concourse import bacc
from concourse.tile import TileContext
from concourse.bass_utils import run_bass_kernel_spmd

BF16 = ml_dtypes.bfloat16

N_CORES = 8
B = 64
C = 768
N = 1024  # H*W = 32*32
NH = 12  # heads
DK = 64
BPC = B // N_CORES  # 8 batches per core
NCHUNK = C // 128  # 6 c-chunks
NTILE = N // 128  # 8 n-tiles
G = 2  # column-strip groups on the PE array for pooling
CHALF = C // G  # 384 output columns per group
# tokens_T layout: [c0..c383, ones, c384..c767, ones] -> 770 columns,
# each group's pooling rhs is a contiguous 385-column slice.
TOKW = C + G
FUSEW = 128 + NH  # fused matmul moving width: transpose cols + logit cols

_CACHE = {}


def _build_module():
    dt = mybir.dt
    nc = bacc.Bacc()

    feats = nc.dram_tensor("features", [BPC, C, N], dt.float32, kind="ExternalInput")
    colsum = nc.dram_tensor("colsum", [BPC, C], dt.float32, kind="ExternalInput")
    rhsc = nc.dram_tensor("rhsc", [128, NCHUNK, FUSEW], dt.bfloat16, kind="ExternalInput")
    posbT = nc.dram_tensor("posbT", [128, NTILE, NH], dt.float32, kind="ExternalInput")
    out = nc.dram_tensor("out", [BPC, NH, C], dt.float32, kind="ExternalOutput")

    with TileContext(nc) as tc:
        with (
            tc.tile_pool(name="consts", bufs=1) as consts,
            tc.tile_pool(name="xpool", bufs=3) as xpool,
            tc.tile_pool(name="tokpool", bufs=2) as tokpool,
            tc.tile_pool(name="sbmisc", bufs=2) as sbmisc,
            tc.tile_pool(name="fpsum", bufs=6, space="PSUM") as fpsum,
            tc.tile_pool(name="ppsum", bufs=2, space="PSUM") as ppsum,
        ):
            rhsc_sb = consts.tile([128, NCHUNK, FUSEW], dt.bfloat16)
            nc.sync.dma_start(out=rhsc_sb, in_=rhsc[:])
            posb_sb = consts.tile([128, NTILE, NH], dt.float32)
            nc.sync.dma_start(out=posb_sb, in_=posbT[:])

            # colsum for all batches, broadcast to the 12 head rows of each
            # group's partition range, loaded once (emitted after batch 0's
            # feature load so it doesn't block startup on the SWDGE queue).
            cs_sb = consts.tile([44, BPC, CHALF], dt.float32)

            def emit_colsum():
                for g in range(G):
                    s = colsum[:, g * CHALF : (g + 1) * CHALF]  # [BPC, 384]
                    bcast = bass.AP(
                        tensor=s.tensor, offset=s.offset, ap=[[0, NH]] + s.ap
                    )
                    nc.gpsimd.dma_start(
                        out=cs_sb[32 * g : 32 * g + NH, :, :], in_=bcast
                    )

            state = {}  # per-batch tiles needed by the delayed (b-1) stages

            def emit_load(b):
                # fp32 -> bf16 cast during the DMA (SWDGE). Batch 0 loads per
                # chunk so the first fused matmul starts as early as possible;
                # later batches use one big DMA per half (SWDGE issue + drain
                # is ~1us per dma_start, so fewer is better once pipelined).
                x_sb = xpool.tile([128, NCHUNK, N], dt.bfloat16, name=f"x_{b}", tag="x")
                src = feats[b].rearrange("(k p) n -> p k n", p=128)
                if b == 0:
                    for k in range(NCHUNK):
                        nc.gpsimd.dma_start(
                            out=x_sb[:, k : k + 1, :], in_=src[:, k : k + 1, :]
                        )
                else:
                    half = NCHUNK // 2
                    for h in range(2):
                        ks = slice(h * half, (h + 1) * half)
                        nc.gpsimd.dma_start(out=x_sb[:, ks, :], in_=src[:, ks, :])
                return x_sb

            def emit_tok_alloc(b):
                tok_sb = tokpool.tile(
                    [128, NTILE, TOKW], dt.bfloat16, name=f"tok_{b}", tag="tok"
                )
                nc.gpsimd.memset(tok_sb[:, :, CHALF : CHALF + 1], 1.0)
                nc.gpsimd.memset(tok_sb[:, :, TOKW - 1 : TOKW], 1.0)
                # per-chunk n-major logit partials, staged in bf16
                lst = sbmisc.tile(
                    [128, NCHUNK, NTILE, NH], dt.bfloat16, name=f"ls_{b}", tag="ls"
                )
                return tok_sb, lst

            # j-triples: a [128, 3, 140] fp32 psum tile fits in one 2KB bank
            TRIPLES = [(0, 3), (3, 3), (6, 2)]

            def emit_fused_chunk(b, k, x_sb, tok_sb, lst):
                # chunk k's transpose column slot in tok
                col = 128 * k if k < 3 else CHALF + 1 + 128 * (k - 3)
                for t, (j0, jw) in enumerate(TRIPLES):
                    fp = fpsum.tile(
                        [128, 3, FUSEW], dt.float32, name=f"fp_{b}_{k}_{t}", tag="fp"
                    )
                    for jj in range(jw):
                        j = j0 + jj
                        nc.tensor.matmul(
                            out=fp[:, jj, :],
                            lhsT=x_sb[:, k, 128 * j : 128 * (j + 1)],
                            rhs=rhsc_sb[:, k, :],
                            start=True,
                            stop=True,
                        )
                    # drain: transpose cols -> tok, logit cols -> staging;
                    # GPSIMD cannot access PSUM, so ACT/DVE alternate, each
                    # taking one big + one small copy per pair of tiles.
                    idx = k * 3 + t
                    dst = tok_sb[:, j0 : j0 + jw, col : col + 128]
                    lso = lst[:, k, j0 : j0 + jw, :]
                    if idx % 2 == 0:
                        nc.scalar.copy(dst, fp[:, 0:jw, 0:128])
                        nc.vector.tensor_copy(lso, fp[:, 0:jw, 128:FUSEW])
                    else:
                        nc.vector.tensor_copy(dst, fp[:, 0:jw, 0:128])
                        nc.scalar.copy(lso, fp[:, 0:jw, 128:FUSEW])

            def emit_expd(b, lst):
                # sum the 6 chunk partials + pos bias on the Pool engine
                # (SBUF-only add tree, no PSUM access, no serial chain)
                s1 = sbmisc.tile(
                    [128, 3, NTILE, NH], dt.float32, name=f"s1_{b}", tag="s1"
                )
                nc.gpsimd.tensor_add(s1[:], lst[:, 0:3, :, :], lst[:, 3:6, :, :])
                s2 = sbmisc.tile([128, NTILE, NH], dt.float32, name=f"s2_{b}", tag="s2")
                nc.gpsimd.tensor_add(s2[:], s1[:, 0, :, :], s1[:, 1, :, :])
                s3 = sbmisc.tile([128, NTILE, NH], dt.float32, name=f"s3_{b}", tag="s3")
                nc.gpsimd.tensor_add(s3[:], s1[:, 2, :, :], posb_sb[:])
                lacc = sbmisc.tile(
                    [128, NTILE, NH], dt.float32, name=f"la_{b}", tag="la"
                )
                nc.gpsimd.tensor_add(lacc[:], s2[:], s3[:])
                exp_sb = sbmisc.tile(
                    [128, NTILE, NH], dt.float32, name=f"exp_{b}", tag="exp"
                )
                nc.scalar.activation(
                    out=exp_sb[:],
                    in_=lacc[:],
                    func=mybir.ActivationFunctionType.Exp,
                )
                d_sb = sbmisc.tile([128, NTILE, NH], dt.bfloat16, name=f"d_{b}", tag="d")
                nc.vector.tensor_scalar_add(d_sb[:], exp_sb[:], -1.0)
                return d_sb

            def emit_pool(b, d_sb, tok_sb):
                pp = ppsum.tile([44, CHALF + 1], dt.float32, name=f"pp_{b}", tag="pp")
                # interleave the two column strips so the PE array runs both
                # concurrently (different col_grp strips).
                for j in range(NTILE):
                    for g in range(G):
                        lo = 32 * g
                        nc.tensor.matmul(
                            out=pp[lo : lo + NH, :],
                            lhsT=d_sb[:, j, :],
                            rhs=tok_sb[:, j, g * (CHALF + 1) : (g + 1) * (CHALF + 1)],
                            start=(j == 0),
                            stop=(j == NTILE - 1),
                        )
                for g in range(G):
                    lo = 32 * g
                    zt = sbmisc.tile([44, 1], dt.float32, name=f"z{g}_{b}", tag=f"z{g}")
                    nc.vector.tensor_scalar_add(
                        zt[lo : lo + NH, :],
                        pp[lo : lo + NH, CHALF : CHALF + 1],
                        float(N),
                    )
                    recip = sbmisc.tile(
                        [44, 1], dt.float32, name=f"r{g}_{b}", tag=f"r{g}"
                    )
                    nc.vector.reciprocal(
                        out=recip[lo : lo + NH, :], in_=zt[lo : lo + NH, :]
                    )
                    num = sbmisc.tile(
                        [44, CHALF], dt.float32, name=f"n{g}_{b}", tag=f"n{g}"
                    )
                    nc.vector.tensor_add(
                        num[lo : lo + NH, :],
                        pp[lo : lo + NH, 0:CHALF],
                        cs_sb[lo : lo + NH, b, :],
                    )
                    osb = sbmisc.tile(
                        [44, CHALF], dt.float32, name=f"o{g}_{b}", tag=f"o{g}"
                    )
                    nc.vector.tensor_scalar_mul(
                        osb[lo : lo + NH, :],
                        num[lo : lo + NH, :],
                        recip[lo : lo + NH, :],
                    )
                    nc.sync.dma_start(
                        out=out[b, :, g * CHALF : (g + 1) * CHALF],
                        in_=osb[lo : lo + NH, :],
                    )

            for b in range(BPC):
                x_sb = emit_load(b)
                if b == 0:
                    emit_colsum()
                tok_sb, lst = emit_tok_alloc(b)
                for k in range(3):
                    emit_fused_chunk(b, k, x_sb, tok_sb, lst)
                if b > 0:
                    # previous batch's pooling between fused bursts: if the
                    # second-half DMA lags, the PE has independent work queued
                    emit_pool(b - 1, state[b - 1]["d"], state[b - 1]["tok"])
                    del state[b - 1]
                for k in range(3, NCHUNK):
                    emit_fused_chunk(b, k, x_sb, tok_sb, lst)
                d_sb = emit_expd(b, lst)
                state[b] = {"d": d_sb, "tok": tok_sb}

            bb = BPC - 1
            emit_pool(bb, state[bb]["d"], state[bb]["tok"])

    nc.compile()
    return nc


def _host_consts(cls, W_k, pos_embed):
    # v_h = W_k[h] @ cls[h] / 8
    V = np.einsum("hcd,hd->hc", W_k.astype(np.float32), cls.astype(np.float32)) / 8.0
    # fused moving operand per chunk k: [I(128) | vT_k], vT_k[p, h] = V[h, 128k+p]
    rhsc = np.zeros((128, NCHUNK, FUSEW), np.float32)
    for k in range(NCHUNK):
        rhsc[:, k, 0:128] = np.eye(128, dtype=np.float32)
        rhsc[:, k, 128:FUSEW] = V[:, 128 * k : 128 * (k + 1)].T
    # pos bias in n-major: posb[n, h] = pos_embed[n] . cls[h] / 8
    posb = pos_embed[0, 0].astype(np.float32) @ (cls.astype(np.float32) / 8.0).T
    posbT = np.ascontiguousarray(posb.reshape(NTILE, 128, NH).transpose(1, 0, 2))
    return rhsc.astype(BF16), posbT.astype(np.float32)


def make_in_maps(features, cls, W_k, pos_embed):
    rhsc, posbT = _host_consts(cls, W_k, pos_embed)
    x = features.reshape(B, C, N)
    colsum = x.sum(axis=2, dtype=np.float64).astype(np.float32)  # [B, C] exact
    in_maps = []
    for core in range(N_CORES):
        sl = slice(core * BPC, (core + 1) * BPC)
        in_maps.append(
            {
                "features": np.ascontiguousarray(x[sl]),
                "colsum": np.ascontiguousarray(colsum[sl]),
                "rhsc": rhsc,
                "posbT": posbT,
            }
        )
    return in_maps


def kernel(features, cls, W_k, pos_embed):
    features = np.asarray(features, dtype=np.float32)
    cls = np.asarray(cls, dtype=np.float32)
    W_k = np.asarray(W_k, dtype=np.float32)
    pos_embed = np.asarray(pos_embed, dtype=np.float32)

    if "nc" not in _CACHE:
        _CACHE["nc"] = _build_module()
    nc = _CACHE["nc"]

    in_maps = make_in_maps(features, cls, W_k, pos_embed)
    res = run_bass_kernel_spmd(nc, in_maps, core_ids=list(range(N_CORES)))
    out = np.concatenate([r["out"] for r in res.results], axis=0)  # [64, 12, 768]
    return np.ascontiguousarray(out.reshape(B, NH * C)).astype(np.float32)


# revision 11
# speedup vs baseline: 1.2084x; 1.2084x over previous
"""Trainium2 Bass kernel for nn_ClsCrossAttention (single-query CLS attention pooling).

Reference computation (per batch b, head h):
    tokens = features[b].reshape(C, H*W).T                  # [N=1024, C=768]
    K      = tokens @ W_k[h] + pos_embed                    # [N, 64]
    logits = K @ cls[h] / 8
    attn   = softmax(logits)
    out    = attn @ tokens                                  # [C]

Restructure (K is never materialized):
    logits[n, h] = tokens[n] . v_h + pos_bias[n, h]
        v_h      = W_k[h] @ cls[h] / 8          (host precomputed, [12, 768])
        pos_bias = pos_embed @ (cls/8)^T        (host precomputed, [1024, 12])
    Logits are ~+-0.02 so softmax needs no max subtraction. With d = exp(l)-1:
        out[h] = (colsum + d_h @ tokens) / (N + sum(d_h))
    colsum = sum_n tokens[n] computed exactly on the host in fp32.

Key PE trick: pooling needs tokens in n-major layout (PE transposes of the
c-major x tiles), and each transpose matmul already loads the x tile
[128c, 128n] as the stationary operand.  Widening the moving operand from
I(128) to [I | vT_k] (140 cols) makes the same matmuls also emit the
chunk-k logit partials in n-major — the separate logits/pos/dT matmuls all
disappear and the fused matmuls stream at ~61ns with LDWEIGHTS fully
hidden.  Per batch the PE runs just 48 fused matmuls + 16 pooling matmuls
(2 column-strip groups for array-level concurrency).

Drains: ACT/DVE alternate copying the transpose columns (PSUM->tok bf16)
and the 12 logit columns (PSUM->bf16 staging); the Pool engine (gpsimd,
which cannot read PSUM but can do SBUF tensor ops) then sums the 6 chunk
partials + pos bias with a small add tree, so no engine carries a serial
accumulation chain.  exp runs on the n-major [128, 96] logits so
d = exp(l)-1 is directly the pooling stationary operand.

DMA: features fp32 -> bf16 cast during DMA (SWDGE), ~3.1 MB fp32 read per
batch; 8 batches per core saturate the ~360 GB/s per-core HBM read budget,
which is the roofline for this kernel.
"""

import sys

sys.path.insert(0, "/opt/trn_rl_repo")

import numpy as np
import ml_dtypes

import concourse.bass as bass
import concourse.mybir as mybir
from concourse import bacc
from concourse.tile import TileContext
from concourse.bass_utils import run_bass_kernel_spmd

BF16 = ml_dtypes.bfloat16

N_CORES = 8
B = 64
C = 768
N = 1024  # H*W = 32*32
NH = 12  # heads
DK = 64
BPC = B // N_CORES  # 8 batches per core
NCHUNK = C // 128  # 6 c-chunks
NTILE = N // 128  # 8 n-tiles
G = 2  # column-strip groups on the PE array for pooling
CHALF = C // G  # 384 output columns per group
# tokens_T layout: [c0..c383, ones, c384..c767, ones] -> 770 columns,
# each group's pooling rhs is a contiguous 385-column slice.
TOKW = C + G
FUSEW = 128 + NH  # fused matmul moving width: transpose cols + logit cols

_CACHE = {}


def _build_module():
    dt = mybir.dt
    nc = bacc.Bacc()

    feats = nc.dram_tensor("features", [BPC, C, N], dt.float32, kind="ExternalInput")
    colsum = nc.dram_tensor("colsum", [BPC, C], dt.float32, kind="ExternalInput")
    rhsc = nc.dram_tensor("rhsc", [128, NCHUNK, FUSEW], dt.bfloat16, kind="ExternalInput")
    posbT = nc.dram_tensor("posbT", [128, NTILE, NH], dt.float32, kind="ExternalInput")
    out = nc.dram_tensor("out", [BPC, NH, C], dt.float32, kind="ExternalOutput")

    with TileContext(nc) as tc:
        with (
            tc.tile_pool(name="consts", bufs=1) as consts,
            tc.tile_pool(name="xpool", bufs=3) as xpool,
            tc.tile_pool(name="tokpool", bufs=2) as tokpool,
            tc.tile_pool(name="sbmisc", bufs=2) as sbmisc,
            tc.tile_pool(name="fpsum", bufs=6, space="PSUM") as fpsum,
            tc.tile_pool(name="ppsum", bufs=2, space="PSUM") as ppsum,
        ):
            rhsc_sb = consts.tile([128, NCHUNK, FUSEW], dt.bfloat16)
            nc.sync.dma_start(out=rhsc_sb, in_=rhsc[:])
            posb_sb = consts.tile([128, NTILE, NH], dt.float32)
            nc.sync.dma_start(out=posb_sb, in_=posbT[:])

            # colsum for all batches, broadcast to the 12 head rows of each
            # group's partition range, loaded once (emitted after batch 0's
            # feature load so it doesn't block startup on the SWDGE queue).
            cs_sb = consts.tile([44, BPC, CHALF], dt.float32)

            def emit_colsum():
                for g in range(G):
                    s = colsum[:, g * CHALF : (g + 1) * CHALF]  # [BPC, 384]
                    bcast = bass.AP(
                        tensor=s.tensor, offset=s.offset, ap=[[0, NH]] + s.ap
                    )
                    nc.gpsimd.dma_start(
                        out=cs_sb[32 * g : 32 * g + NH, :, :], in_=bcast
                    )

            state = {}  # per-batch tiles needed by the delayed (b-1) stages

            def emit_load(b):
                # fp32 -> bf16 cast during the DMA (SWDGE). Batch 0 loads per
                # chunk so the first fused matmul starts as early as possible;
                # later batches use one big DMA per half (SWDGE issue + drain
                # is ~1us per dma_start, so fewer is better once pipelined).
                x_sb = xpool.tile([128, NCHUNK, N], dt.bfloat16, name=f"x_{b}", tag="x")
                src = feats[b].rearrange("(k p) n -> p k n", p=128)
                if b == 0:
                    for k in range(NCHUNK):
                        nc.gpsimd.dma_start(
                            out=x_sb[:, k : k + 1, :], in_=src[:, k : k + 1, :]
                        )
                else:
                    half = NCHUNK // 2
                    for h in range(2):
                        ks = slice(h * half, (h + 1) * half)
                        nc.gpsimd.dma_start(out=x_sb[:, ks, :], in_=src[:, ks, :])
                return x_sb

            def emit_tok_alloc(b):
                tok_sb = tokpool.tile(
                    [128, NTILE, TOKW], dt.bfloat16, name=f"tok_{b}", tag="tok"
                )
                nc.gpsimd.memset(tok_sb[:, :, CHALF : CHALF + 1], 1.0)
                nc.gpsimd.memset(tok_sb[:, :, TOKW - 1 : TOKW], 1.0)
                lacc = sbmisc.tile(
                    [128, NTILE, NH], dt.float32, name=f"la_{b}", tag="la"
                )
                return tok_sb, lacc

            # j-triples: a [128, 3, 140] fp32 psum tile fits in one 2KB bank
            TRIPLES = [(0, 3), (3, 3), (6, 2)]

            def emit_fused_chunk(b, k, x_sb, tok_sb, lacc):
                # chunk k's transpose column slot in tok
                col = 128 * k if k < 3 else CHALF + 1 + 128 * (k - 3)
                for t, (j0, jw) in enumerate(TRIPLES):
                    fp = fpsum.tile(
                        [128, 3, FUSEW], dt.float32, name=f"fp_{b}_{k}_{t}", tag="fp"
                    )
                    for jj in range(jw):
                        j = j0 + jj
                        nc.tensor.matmul(
                            out=fp[:, jj, :],
                            lhsT=x_sb[:, k, 128 * j : 128 * (j + 1)],
                            rhs=rhsc_sb[:, k, :],
                            start=True,
                            stop=True,
                        )
                    # drain: transpose cols -> tok (ACT-heavy split: ACT is
                    # otherwise idle, DVE carries the accumulate + epilogue),
                    # logit cols accumulated into lacc on DVE (pos bias init).
                    idx = k * 3 + t
                    dst = tok_sb[:, j0 : j0 + jw, col : col + 128]
                    if idx % 3 == 2:
                        nc.vector.tensor_copy(dst, fp[:, 0:jw, 0:128])
                    else:
                        nc.scalar.copy(dst, fp[:, 0:jw, 0:128])
                    if k == 0:
                        nc.vector.tensor_add(
                            lacc[:, j0 : j0 + jw, :],
                            fp[:, 0:jw, 128:FUSEW],
                            posb_sb[:, j0 : j0 + jw, :],
                        )
                    else:
                        nc.vector.tensor_add(
                            lacc[:, j0 : j0 + jw, :],
                            lacc[:, j0 : j0 + jw, :],
                            fp[:, 0:jw, 128:FUSEW],
                        )

            def emit_expd(b, lacc):
                exp_sb = sbmisc.tile(
                    [128, NTILE, NH], dt.float32, name=f"exp_{b}", tag="exp"
                )
                nc.scalar.activation(
                    out=exp_sb[:],
                    in_=lacc[:],
                    func=mybir.ActivationFunctionType.Exp,
                )
                d_sb = sbmisc.tile([128, NTILE, NH], dt.bfloat16, name=f"d_{b}", tag="d")
                nc.vector.tensor_scalar_add(d_sb[:], exp_sb[:], -1.0)
                return d_sb

            def emit_pool(b, d_sb, tok_sb):
                pp = ppsum.tile([44, CHALF + 1], dt.float32, name=f"pp_{b}", tag="pp")
                # interleave the two column strips so the PE array runs both
                # concurrently (different col_grp strips).
                for j in range(NTILE):
                    for g in range(G):
                        lo = 32 * g
                        nc.tensor.matmul(
                            out=pp[lo : lo + NH, :],
                            lhsT=d_sb[:, j, :],
                            rhs=tok_sb[:, j, g * (CHALF + 1) : (g + 1) * (CHALF + 1)],
                            start=(j == 0),
                            stop=(j == NTILE - 1),
                        )
                for g in range(G):
                    lo = 32 * g
                    zt = sbmisc.tile([44, 1], dt.float32, name=f"z{g}_{b}", tag=f"z{g}")
                    nc.vector.tensor_scalar_add(
                        zt[lo : lo + NH, :],
                        pp[lo : lo + NH, CHALF : CHALF + 1],
                        float(N),
                    )
                    recip = sbmisc.tile(
                        [44, 1], dt.float32, name=f"r{g}_{b}", tag=f"r{g}"
                    )
                    nc.vector.reciprocal(
                        out=recip[lo : lo + NH, :], in_=zt[lo : lo + NH, :]
                    )
                    num = sbmisc.tile(
                        [44, CHALF], dt.float32, name=f"n{g}_{b}", tag=f"n{g}"
                    )
                    nc.vector.tensor_add(
                        num[lo : lo + NH, :],
                        pp[lo : lo + NH, 0:CHALF],
                        cs_sb[lo : lo + NH, b, :],
                    )
                    osb = sbmisc.tile(
                        [44, CHALF], dt.float32, name=f"o{g}_{b}", tag=f"o{g}"
                    )
                    nc.vector.tensor_scalar_mul(
                        osb[lo : lo + NH, :],
                        num[lo : lo + NH, :],
                        recip[lo : lo + NH, :],
                    )
                    nc.sync.dma_start(
                        out=out[b, :, g * CHALF : (g + 1) * CHALF],
                        in_=osb[lo : lo + NH, :],
                    )

            # software-pipelined: the NEXT batch's SWDGE load is issued at the
            # top of each period, before any other gpsimd-queue work, so the
            # DMA rings never starve behind compute-dependent instructions.
            x_tiles = {0: emit_load(0)}
            emit_colsum()
            for b in range(BPC):
                x_sb = x_tiles.pop(b)
                if b + 1 < BPC:
                    x_tiles[b + 1] = emit_load(b + 1)
                tok_sb, lacc = emit_tok_alloc(b)
                for k in range(3):
                    emit_fused_chunk(b, k, x_sb, tok_sb, lacc)
                if b > 0:
                    # previous batch's pooling between fused bursts: if the
                    # second-half DMA lags, the PE has independent work queued
                    emit_pool(b - 1, state[b - 1]["d"], state[b - 1]["tok"])
                    del state[b - 1]
                for k in range(3, NCHUNK):
                    emit_fused_chunk(b, k, x_sb, tok_sb, lacc)
                d_sb = emit_expd(b, lacc)
                state[b] = {"d": d_sb, "tok": tok_sb}

            bb = BPC - 1
            emit_pool(bb, state[bb]["d"], state[bb]["tok"])

    nc.compile()
    return nc


def _host_consts(cls, W_k, pos_embed):
    # v_h = W_k[h] @ cls[h] / 8
    V = np.einsum("hcd,hd->hc", W_k.astype(np.float32), cls.astype(np.float32)) / 8.0
    # fused moving operand per chunk k: [I(128) | vT_k], vT_k[p, h] = V[h, 128k+p]
    rhsc = np.zeros((128, NCHUNK, FUSEW), np.float32)
    for k in range(NCHUNK):
        rhsc[:, k, 0:128] = np.eye(128, dtype=np.float32)
        rhsc[:, k, 128:FUSEW] = V[:, 128 * k : 128 * (k + 1)].T
    # pos bias in n-major: posb[n, h] = pos_embed[n] . cls[h] / 8
    posb = pos_embed[0, 0].astype(np.float32) @ (cls.astype(np.float32) / 8.0).T
    posbT = np.ascontiguousarray(posb.reshape(NTILE, 128, NH).transpose(1, 0, 2))
    return rhsc.astype(BF16), posbT.astype(np.float32)


def make_in_maps(features, cls, W_k, pos_embed):
    rhsc, posbT = _host_consts(cls, W_k, pos_embed)
    x = features.reshape(B, C, N)
    colsum = x.sum(axis=2, dtype=np.float64).astype(np.float32)  # [B, C] exact
    in_maps = []
    for core in range(N_CORES):
        sl = slice(core * BPC, (core + 1) * BPC)
        in_maps.append(
            {
                "features": np.ascontiguousarray(x[sl]),
                "colsum": np.ascontiguousarray(colsum[sl]),
                "rhsc": rhsc,
                "posbT": posbT,
            }
        )
    return in_maps


def kernel(features, cls, W_k, pos_embed):
    features = np.asarray(features, dtype=np.float32)
    cls = np.asarray(cls, dtype=np.float32)
    W_k = np.asarray(W_k, dtype=np.float32)
    pos_embed = np.asarray(pos_embed, dtype=np.float32)

    if "nc" not in _CACHE:
        _CACHE["nc"] = _build_module()
    nc = _CACHE["nc"]

    in_maps = make_in_maps(features, cls, W_k, pos_embed)
    res = run_bass_kernel_spmd(nc, in_maps, core_ids=list(range(N_CORES)))
    out = np.concatenate([r["out"] for r in res.results], axis=0)  # [64, 12, 768]
    return np.ascontiguousarray(out.reshape(B, NH * C)).astype(np.float32)


# revision 14
# speedup vs baseline: 1.2715x; 1.0522x over previous
"""Trainium2 Bass kernel for nn_ClsCrossAttention (single-query CLS attention pooling).

Reference computation (per batch b, head h):
    tokens = features[b].reshape(C, H*W).T                  # [N=1024, C=768]
    K      = tokens @ W_k[h] + pos_embed                    # [N, 64]
    logits = K @ cls[h] / 8
    attn   = softmax(logits)
    out    = attn @ tokens                                  # [C]

Restructure (K is never materialized):
    logits[n, h] = tokens[n] . v_h + pos_bias[n, h]
        v_h      = W_k[h] @ cls[h] / 8          (host precomputed, [12, 768])
        pos_bias = pos_embed @ (cls/8)^T        (host precomputed, [1024, 12])
    Logits are ~+-0.02 so softmax needs no max subtraction. With d = exp(l)-1:
        out[h] = (colsum + d_h @ tokens) / (N + sum(d_h))
    colsum = sum_n tokens[n] computed exactly on the host in fp32.

Key PE trick: pooling needs tokens in n-major layout (PE transposes of the
c-major x tiles), and each transpose matmul already loads the x tile
[128c, 128n] as the stationary operand.  Widening the moving operand from
I(128) to [I | vT_k] (140 cols) makes the same matmuls also emit the
chunk-k logit partials in n-major — the separate logits/pos/dT matmuls all
disappear and the fused matmuls stream at ~61ns with LDWEIGHTS fully
hidden.  Per batch the PE runs just 48 fused matmuls + 16 pooling matmuls
(2 column-strip groups for array-level concurrency).

Drains: ACT/DVE alternate copying the transpose columns (PSUM->tok bf16)
and the 12 logit columns (PSUM->bf16 staging); the Pool engine (gpsimd,
which cannot read PSUM but can do SBUF tensor ops) then sums the 6 chunk
partials + pos bias with a small add tree, so no engine carries a serial
accumulation chain.  exp runs on the n-major [128, 96] logits so
d = exp(l)-1 is directly the pooling stationary operand.

DMA: features fp32 -> bf16 cast during DMA (SWDGE), ~3.1 MB fp32 read per
batch; 8 batches per core saturate the ~360 GB/s per-core HBM read budget,
which is the roofline for this kernel.
"""

import sys

sys.path.insert(0, "/opt/trn_rl_repo")

import numpy as np
import ml_dtypes

import concourse.bass as bass
import concourse.mybir as mybir
from concourse import bacc
from concourse.tile import TileContext
from concourse.bass_utils import run_bass_kernel_spmd

BF16 = ml_dtypes.bfloat16

N_CORES = 8
B = 64
C = 768
N = 1024  # H*W = 32*32
NH = 12  # heads
DK = 64
BPC = B // N_CORES  # 8 batches per core
NCHUNK = C // 128  # 6 c-chunks
NTILE = N // 128  # 8 n-tiles
G = 2  # column-strip groups on the PE array for pooling
CHALF = C // G  # 384 output columns per group
# tokens_T layout: [c0..c383, ones, c384..c767, ones] -> 770 columns,
# each group's pooling rhs is a contiguous 385-column slice.
TOKW = C + G
FUSEW = 128 + NH  # fused matmul moving width: transpose cols + logit cols

_CACHE = {}


def _build_module():
    dt = mybir.dt
    nc = bacc.Bacc()

    feats = nc.dram_tensor("features", [BPC, C, N], dt.float32, kind="ExternalInput")
    colsum = nc.dram_tensor("colsum", [BPC, C], dt.float32, kind="ExternalInput")
    rhsc = nc.dram_tensor("rhsc", [128, NCHUNK, FUSEW], dt.bfloat16, kind="ExternalInput")
    posbT = nc.dram_tensor("posbT", [128, NTILE, NH], dt.float32, kind="ExternalInput")
    out = nc.dram_tensor("out", [BPC, NH, C], dt.float32, kind="ExternalOutput")

    with TileContext(nc) as tc:
        with (
            tc.tile_pool(name="consts", bufs=1) as consts,
            tc.tile_pool(name="xpool", bufs=3) as xpool,
            tc.tile_pool(name="tokpool", bufs=2) as tokpool,
            tc.tile_pool(name="sbmisc", bufs=2) as sbmisc,
            tc.tile_pool(name="fpsum", bufs=7, space="PSUM") as fpsum,
            tc.tile_pool(name="ppsum", bufs=1, space="PSUM") as ppsum,
        ):
            rhsc_sb = consts.tile([128, NCHUNK, FUSEW], dt.bfloat16)
            nc.sync.dma_start(out=rhsc_sb, in_=rhsc[:])
            posb_sb = consts.tile([128, NTILE, NH], dt.float32)
            nc.sync.dma_start(out=posb_sb, in_=posbT[:])

            # colsum for all batches, broadcast to the 12 head rows of each
            # group's partition range, loaded once (emitted after batch 0's
            # feature load so it doesn't block startup on the SWDGE queue).
            cs_sb = consts.tile([44, BPC, CHALF], dt.float32)

            def emit_colsum():
                # strip 0 fills rows 0-31 (12-31 are dummies) so the merged
                # 44-row epilogue ops never touch undefined SBUF
                rows = [(0, 32), (32, NH)]
                for g in range(G):
                    lo, nr = rows[g]
                    s = colsum[:, g * CHALF : (g + 1) * CHALF]  # [BPC, 384]
                    bcast = bass.AP(
                        tensor=s.tensor, offset=s.offset, ap=[[0, nr]] + s.ap
                    )
                    nc.gpsimd.dma_start(
                        out=cs_sb[lo : lo + nr, :, :], in_=bcast
                    )

            state = {}  # per-batch tiles needed by the delayed (b-1) stages

            def emit_load(b):
                # fp32 -> bf16 cast during the DMA (SWDGE). Batch 0 loads per
                # chunk so the first fused matmul starts as early as possible;
                # later batches use one big DMA per half (SWDGE issue + drain
                # is ~1us per dma_start, so fewer is better once pipelined).
                x_sb = xpool.tile([128, NCHUNK, N], dt.bfloat16, name=f"x_{b}", tag="x")
                src = feats[b].rearrange("(k p) n -> p k n", p=128)
                if b == 0:
                    for k in range(NCHUNK):
                        nc.gpsimd.dma_start(
                            out=x_sb[:, k : k + 1, :], in_=src[:, k : k + 1, :]
                        )
                else:
                    half = NCHUNK // 2
                    for h in range(2):
                        ks = slice(h * half, (h + 1) * half)
                        nc.gpsimd.dma_start(out=x_sb[:, ks, :], in_=src[:, ks, :])
                return x_sb

            def emit_tok_alloc(b):
                tok_sb = tokpool.tile(
                    [128, NTILE, TOKW], dt.bfloat16, name=f"tok_{b}", tag="tok"
                )
                nc.gpsimd.memset(tok_sb[:, :, CHALF : CHALF + 1], 1.0)
                nc.gpsimd.memset(tok_sb[:, :, TOKW - 1 : TOKW], 1.0)
                lacc = sbmisc.tile(
                    [128, NTILE, NH], dt.float32, name=f"la_{b}", tag="la"
                )
                return tok_sb, lacc

            # j-triples: a [128, 3, 140] fp32 psum tile fits in one 2KB bank
            TRIPLES = [(0, 3), (3, 3), (6, 2)]

            def emit_fused_chunk(b, k, x_sb, tok_sb, lacc):
                # chunk k's transpose column slot in tok
                col = 128 * k if k < 3 else CHALF + 1 + 128 * (k - 3)
                for t, (j0, jw) in enumerate(TRIPLES):
                    fp = fpsum.tile(
                        [128, 3, FUSEW], dt.float32, name=f"fp_{b}_{k}_{t}", tag="fp"
                    )
                    for jj in range(jw):
                        j = j0 + jj
                        nc.tensor.matmul(
                            out=fp[:, jj, :],
                            lhsT=x_sb[:, k, 128 * j : 128 * (j + 1)],
                            rhs=rhsc_sb[:, k, :],
                            start=True,
                            stop=True,
                        )
                    # drain: transpose cols -> tok (ACT-heavy split: ACT is
                    # otherwise idle, DVE carries the accumulate + epilogue),
                    # logit cols accumulated into lacc on DVE (pos bias init).
                    idx = k * 3 + t
                    dst = tok_sb[:, j0 : j0 + jw, col : col + 128]
                    if idx % 3 == 2:
                        nc.vector.tensor_copy(dst, fp[:, 0:jw, 0:128])
                    else:
                        nc.scalar.copy(dst, fp[:, 0:jw, 0:128])
                    if k == 0:
                        nc.vector.tensor_add(
                            lacc[:, j0 : j0 + jw, :],
                            fp[:, 0:jw, 128:FUSEW],
                            posb_sb[:, j0 : j0 + jw, :],
                        )
                    else:
                        nc.vector.tensor_add(
                            lacc[:, j0 : j0 + jw, :],
                            lacc[:, j0 : j0 + jw, :],
                            fp[:, 0:jw, 128:FUSEW],
                        )

            def emit_expd(b, lacc):
                exp_sb = sbmisc.tile(
                    [128, NTILE, NH], dt.float32, name=f"exp_{b}", tag="exp"
                )
                nc.scalar.activation(
                    out=exp_sb[:],
                    in_=lacc[:],
                    func=mybir.ActivationFunctionType.Exp,
                )
                d_sb = sbmisc.tile([128, NTILE, NH], dt.bfloat16, name=f"d_{b}", tag="d")
                nc.vector.tensor_scalar_add(d_sb[:], exp_sb[:], -1.0)
                return d_sb

            def emit_pool(b, d_sb, tok_sb):
                pp = ppsum.tile([44, CHALF + 1], dt.float32, name=f"pp_{b}", tag="pp")
                # interleave the two column strips so the PE array runs both
                # concurrently (different col_grp strips).
                for j in range(NTILE):
                    for g in range(G):
                        lo = 32 * g
                        nc.tensor.matmul(
                            out=pp[lo : lo + NH, :],
                            lhsT=d_sb[:, j, :],
                            rhs=tok_sb[:, j, g * (CHALF + 1) : (g + 1) * (CHALF + 1)],
                            start=(j == 0),
                            stop=(j == NTILE - 1),
                        )
                # merged 44-row epilogue: both strips in one op each (rows
                # 12-31 are dummies — unwritten psum reads as junk, but those
                # rows are never DMA'd out)
                zt = sbmisc.tile([44, 1], dt.float32, name=f"z_{b}", tag="z")
                nc.vector.tensor_scalar_add(
                    zt[:], pp[:, CHALF : CHALF + 1], float(N)
                )
                recip = sbmisc.tile([44, 1], dt.float32, name=f"r_{b}", tag="r")
                nc.vector.reciprocal(out=recip[:], in_=zt[:])
                num = sbmisc.tile([44, CHALF], dt.float32, name=f"n_{b}", tag="n")
                nc.vector.tensor_add(num[:], pp[:, 0:CHALF], cs_sb[:, b, :])
                osb = sbmisc.tile([44, CHALF], dt.float32, name=f"o_{b}", tag="o")
                nc.vector.tensor_scalar_mul(osb[:], num[:], recip[:])
                for g in range(G):
                    lo = 32 * g
                    nc.sync.dma_start(
                        out=out[b, :, g * CHALF : (g + 1) * CHALF],
                        in_=osb[lo : lo + NH, :],
                    )

            # software-pipelined: the NEXT batch's SWDGE load is issued at the
            # top of each period, before any other gpsimd-queue work, so the
            # DMA rings never starve behind compute-dependent instructions.
            x_tiles = {0: emit_load(0)}
            emit_colsum()
            for b in range(BPC):
                x_sb = x_tiles.pop(b)
                if b + 1 < BPC:
                    x_tiles[b + 1] = emit_load(b + 1)
                tok_sb, lacc = emit_tok_alloc(b)
                for k in range(3):
                    emit_fused_chunk(b, k, x_sb, tok_sb, lacc)
                if b > 0:
                    # previous batch's pooling between fused bursts: if the
                    # second-half DMA lags, the PE has independent work queued
                    emit_pool(b - 1, state[b - 1]["d"], state[b - 1]["tok"])
                    del state[b - 1]
                for k in range(3, NCHUNK):
                    emit_fused_chunk(b, k, x_sb, tok_sb, lacc)
                d_sb = emit_expd(b, lacc)
                state[b] = {"d": d_sb, "tok": tok_sb}

            bb = BPC - 1
            emit_pool(bb, state[bb]["d"], state[bb]["tok"])

    nc.compile()
    return nc


def _host_consts(cls, W_k, pos_embed):
    # v_h = W_k[h] @ cls[h] / 8
    V = np.einsum("hcd,hd->hc", W_k.astype(np.float32), cls.astype(np.float32)) / 8.0
    # fused moving operand per chunk k: [I(128) | vT_k], vT_k[p, h] = V[h, 128k+p]
    rhsc = np.zeros((128, NCHUNK, FUSEW), np.float32)
    for k in range(NCHUNK):
        rhsc[:, k, 0:128] = np.eye(128, dtype=np.float32)
        rhsc[:, k, 128:FUSEW] = V[:, 128 * k : 128 * (k + 1)].T
    # pos bias in n-major: posb[n, h] = pos_embed[n] . cls[h] / 8
    posb = pos_embed[0, 0].astype(np.float32) @ (cls.astype(np.float32) / 8.0).T
    posbT = np.ascontiguousarray(posb.reshape(NTILE, 128, NH).transpose(1, 0, 2))
    return rhsc.astype(BF16), posbT.astype(np.float32)


def make_in_maps(features, cls, W_k, pos_embed):
    rhsc, posbT = _host_consts(cls, W_k, pos_embed)
    x = features.reshape(B, C, N)
    colsum = x.sum(axis=2, dtype=np.float64).astype(np.float32)  # [B, C] exact
    in_maps = []
    for core in range(N_CORES):
        sl = slice(core * BPC, (core + 1) * BPC)
        in_maps.append(
            {
                "features": np.ascontiguousarray(x[sl]),
                "colsum": np.ascontiguousarray(colsum[sl]),
                "rhsc": rhsc,
                "posbT": posbT,
            }
        )
    return in_maps


def kernel(features, cls, W_k, pos_embed):
    features = np.asarray(features, dtype=np.float32)
    cls = np.asarray(cls, dtype=np.float32)
    W_k = np.asarray(W_k, dtype=np.float32)
    pos_embed = np.asarray(pos_embed, dtype=np.float32)

    if "nc" not in _CACHE:
        _CACHE["nc"] = _build_module()
    nc = _CACHE["nc"]

    in_maps = make_in_maps(features, cls, W_k, pos_embed)
    res = run_bass_kernel_spmd(nc, in_maps, core_ids=list(range(N_CORES)))
    out = np.concatenate([r["out"] for r in res.results], axis=0)  # [64, 12, 768]
    return np.ascontiguousarray(out.reshape(B, NH * C)).astype(np.float32)
